# revision 43
# baseline (speedup 1.0000x reference)
"""Multi-head attention (B=2, N=2048, C=1024, H=16, D=64) on 8 Trainium2 cores.

Sharding: core c handles batch b=c//4 and heads [4r, 4r+4) where r=c%4,
processed as two head-pairs p=0,1 (local heads 2p, 2p+1).

Schedule (single software pipeline, ordered to keep ACT's softmax-exp stream
and the PE matmul stream running concurrently):
  A(p): qkv matmuls + LayerNorm + DMA-transpose into [d, n] layout, per pair.
  B(p): attention per pair; scores for two jt blocks at a time feed one wide
        [128, 2048] exp; attn@V accumulates with a ones-column for sumexp.
  A(0) leads in; B(0) is interleaved with A(1); B(1) is interleaved with the
  pair-0 half of the output projection (C0); the pair-1 half (C1) runs after
  the second AllToAll.
Per-head attention output is redistributed head->sequence via one AllToAll
per pair; core g projects rows [g*256, (g+1)*256) of both batches.

rstd for LayerNorm is computed as Exp(-0.5*Ln(var+eps)) so the ACT engine
stays on the natural_log_exp table set used by the softmax exp (a Sqrt would
force a ~2.7us table reload on every interleaved LN/exp switch).
"""
import os
import numpy as np

B, N, C = 2, 2048, 1024
H, D = 16, 64
LN_EPS = 1e-6
N_CORES = 8
IQ = 512          # i-range per attention inner block (PSUM-sized)

_CACHE = {}


def _install_trace_shim():
    """Recreate the missing antenv.axon_hooks module so trace=True works."""
    import sys, types
    if "antenv.axon_hooks" in sys.modules:
        return
    try:
        import antenv
        mod = types.ModuleType("antenv.axon_hooks")
        mod._hook = None
        mod.set_axon_ntff_profile_hook = lambda h: setattr(mod, "_hook", h)
        mod.get_axon_ntff_profile_hook = lambda: mod._hook
        sys.modules["antenv.axon_hooks"] = mod
        antenv.axon_hooks = mod
        from trn_agent_boot.trn_boot import _ntff_profile_via_ctypes
        mod._hook = _ntff_profile_via_ctypes("/opt/axon/libaxon_pjrt.so")
    except Exception:
        pass


def _build():
    import concourse.bacc as bacc
    import concourse.bass as bass
    import concourse.tile as tile
    from concourse import mybir
    from contextlib import ExitStack

    f32 = mybir.dt.float32
    mdt = mybir.dt.bfloat16

    AP = bass.AP
    nc = bacc.Bacc("TRN2", target_bir_lowering=False, debug=False,
                   num_devices=N_CORES)

    # ---- DRAM I/O (per-core shards prepared on host) ----
    xT_d = nc.dram_tensor("xT", [C, N], mdt, kind="ExternalInput")        # x[b].T
    w_d = nc.dram_tensor("w_all", [C, 2, 384], mdt, kind="ExternalInput") # [pair][q128|k128|v128]
    bias_d = nc.dram_tensor("bias_all", [2, 384], f32, kind="ExternalInput")
    aff_d = nc.dram_tensor("aff", [4, 128], f32, kind="ExternalInput")    # qsc2,qbi2,ksc2,kbi2
    wproj_d = nc.dram_tensor("wproj", [C, C], mdt, kind="ExternalInput")
    bproj_d = nc.dram_tensor("bproj", [C], f32, kind="ExternalInput")
    out_d = nc.dram_tensor("out_part", [B, 256, C], f32, kind="ExternalOutput")

    def bcast(dram_handle, n_parts, free):
        ap = dram_handle.ap()
        return AP(tensor=ap.tensor, offset=0, ap=[[0, n_parts], [1, free]])

    groups = [[0, 1, 2, 3, 4, 5, 6, 7]]
    Exp = mybir.ActivationFunctionType.Exp
    Ln = mybir.ActivationFunctionType.Ln
    Alu = mybir.AluOpType

    with tile.TileContext(nc) as tc:
        with ExitStack() as ctx:
            g = ctx.enter_context(tc.tile_pool(name="globals", bufs=1))
            dram = ctx.enter_context(tc.tile_pool(name="dram", bufs=1, space="DRAM"))

            # ---- persistent tiles ----
            from concourse.masks import make_identity
            ident_f = g.tile([128, 128], f32, tag="ident_f")
            make_identity(nc, ident_f)
            identity = g.tile([128, 128], mdt, tag="ident")
            nc.vector.tensor_copy(out=identity, in_=ident_f)
            eps_t = g.tile([128, 1], f32, tag="eps")
            nc.vector.memset(eps_t, LN_EPS)
            bias_bc = g.tile([128, 2, 384], f32, tag="bias_bc")
            aff_sb = g.tile([128, 4], f32, tag="aff_sb")
            bproj_bc = g.tile([128, C], f32, tag="bproj_bc")

            xT = g.tile([128, 8, N], mdt, tag="xT")
            w_all = g.tile([128, 8, 2, 384], mdt, tag="w_all")
            wp_sb = g.tile([128, 8, C], mdt, tag="wp_sb")

            q2 = g.tile([128, 2, N], mdt, tag="q2")     # [2heads x 64d, pair, n]
            k2 = g.tile([128, 2, N], mdt, tag="k2")
            v_all = g.tile([128, 2, 16, 2, D + 1], mdt, tag="v_all")  # [n, pair, nt, hp, d+1]
            outT2 = g.tile([128, 2, N], mdt, tag="outT2")  # [2hp x 64d, pair, i]
            o_sb = g.tile([128, 4, C], f32, tag="o_sb")    # C0 partials per (bb*2+mt)

            # ones column for the sumexp row of attn@V
            nc.vector.memset(v_all[:, :, :, :, D:D + 1], 1.0)

            # ---- input DMAs ----
            # small tensors + x on scalar queue, qkv weights on sync; the
            # (late-needed) projection weights and bias are deferred below
            nc.scalar.dma_start(out=bias_bc, in_=bcast(bias_d, 128, 768))
            nc.scalar.dma_start(
                out=aff_sb,
                in_=AP(tensor=aff_d.ap().tensor, offset=0, ap=[[1, 128], [128, 4]]))
            for kc in range(8):
                nc.sync.dma_start(out=w_all[:, kc], in_=w_d.ap()[kc * 128:(kc + 1) * 128])
                nc.scalar.dma_start(out=xT[:, kc, 0:512],
                                    in_=xT_d.ap()[kc * 128:(kc + 1) * 128, 0:512])
            for kc in range(8):
                nc.sync.dma_start(out=xT[:, kc, 512:1024],
                                  in_=xT_d.ap()[kc * 128:(kc + 1) * 128, 512:1024])
                nc.scalar.dma_start(out=xT[:, kc, 1024:2048],
                                    in_=xT_d.ap()[kc * 128:(kc + 1) * 128, 1024:2048])

            # collective staging
            cc_in = [dram.tile([8, 128, 256], mdt, name=f"cc_in{p}") for p in range(2)]
            cc_out = [dram.tile([8, 128, 256], mdt, name=f"cc_out{p}") for p in range(2)]
            r_dram = nc.dram_tensor("r_stage", [16, IQ], f32).ap()
            r_dram2 = nc.dram_tensor("r_stage2", [16, IQ], f32).ap()

            # tiny dummy collective to wake the CC cores early so the real
            # AllToAlls run warm (safe now that no DMA-transposes exist to be
            # serialized against an in-flight collective)
            cc_wi = dram.tile([8, 16], f32, name="cc_wi")
            cc_wo = dram.tile([8, 16], f32, name="cc_wo")
            warm_src = g.tile([8, 16], f32, tag="warm_src")
            nc.vector.memset(warm_src, 0.0)
            nc.gpsimd.dma_start(out=cc_wi, in_=warm_src)
            nc.gpsimd.collective_compute(
                "AllToAll", Alu.bypass, replica_groups=groups,
                ins=[cc_wi.opt()], outs=[cc_wo.opt()])



            # ---- pools ----
            psS = ctx.enter_context(tc.tile_pool(name="psS", bufs=2, space="PSUM"))
            ptp = ctx.enter_context(tc.tile_pool(name="ptp", bufs=2))
            nrm = ctx.enter_context(tc.tile_pool(name="nrm", bufs=3))

            # A-phase pools (and B(0)'s single-buffered ps_o) opened last so
            # they can be popped (LIFO) to free PSUM banks for B(1)/stage C
            actx = ExitStack()
            psA = actx.enter_context(tc.tile_pool(name="psA", bufs=1, space="PSUM"))
            psT = actx.enter_context(tc.tile_pool(name="psT", bufs=1, space="PSUM"))
            psO = actx.enter_context(tc.tile_pool(name="psO", bufs=1, space="PSUM"))
            sbA = actx.enter_context(tc.tile_pool(name="sbA", bufs=8))
            stA = actx.enter_context(tc.tile_pool(name="stA", bufs=2))

            # =========== stage A: per-nt matmul block + per-4-nt finish ===========
            A_state = {}

            def emit_A_mm(pair, nt):
                i = nt % 4
                if i == 0:
                    A_state[pair] = {
                        "mv": stA.tile([128, 4, 4, 2], f32, tag="mv",
                                       name=f"mv{pair}_{nt // 4}"),
                        "qkn": {},
                    }
                mv = A_state[pair]["mv"]
                ps = psA.tile([128, 512], f32, tag="psA", name=f"psA{pair}_{nt}")
                for kc in range(8):
                    nc.tensor.matmul(ps[:, 0:384],
                                     xT[:, kc, nt * 128:(nt + 1) * 128],
                                     w_all[:, kc, pair, :],
                                     start=(kc == 0), stop=(kc == 7))
                qk_sb = sbA.tile([128, 256], mdt, tag="qk_sb", name=f"qk{pair}_{nt}")
                nc.vector.tensor_tensor(out=qk_sb, in0=ps[:, 0:256],
                                        in1=bias_bc[:, pair, 0:256], op=Alu.add)
                nc.vector.tensor_tensor(out=v_all[:, pair, nt, :, 0:D],
                                        in0=ps[:, 256:384].rearrange("p (h d) -> p h d", h=2),
                                        in1=bias_bc[:, pair, 256:384].rearrange("p (h d) -> p h d", h=2),
                                        op=Alu.add)
                st = stA.tile([128, 4, 6], f32, tag="st", name=f"st{pair}_{nt}")
                for gi in range(4):
                    nc.vector.bn_stats(out=st[:, gi, :],
                                       in_=qk_sb[:, gi * D:(gi + 1) * D])
                    nc.vector.bn_aggr(out=mv[:, i, gi, :], in_=st[:, gi, :])
                A_state[pair]["qkn"][i] = qk_sb

            def emit_A_fin(pair, s):
                mv = A_state[pair]["mv"]
                qkn = A_state[pair]["qkn"]
                rstd = stA.tile([128, 4, 4], f32, tag="rstd", name=f"rstd{pair}_{s}")
                # rstd = 1/sqrt(var+eps) on DVE (fast inverse sqrt + 1 Newton
                # step) so ACT runs nothing but Exp -> one table set, no reloads
                ve = stA.tile([128, 16], f32, tag="ve", name=f"ve{pair}_{s}")
                var_view = mv[:, :, :, 1:2].rearrange("p a g o -> p (a g o)")
                nc.vector.tensor_scalar(out=ve, in0=var_view, scalar1=LN_EPS,
                                        scalar2=None, op0=Alu.add)
                y0 = stA.tile([128, 16], f32, tag="y0", name=f"y0{pair}_{s}")
                y0_i = y0.bitcast(mybir.dt.int32)
                nc.vector.tensor_scalar(out=y0_i, in0=ve.bitcast(mybir.dt.int32),
                                        scalar1=1, scalar2=None,
                                        op0=Alu.logical_shift_right)
                nc.vector.tensor_scalar(out=y0_i, in0=y0_i, scalar1=-1,
                                        scalar2=None, op0=Alu.bitwise_xor)
                nc.vector.tensor_scalar(out=y0_i, in0=y0_i, scalar1=0x5f3759df + 1,
                                        scalar2=None, op0=Alu.add)
                rsf = rstd.rearrange("p a g -> p (a g)")
                nc.vector.tensor_tensor(out=rsf, in0=y0, in1=y0, op=Alu.mult)
                nc.vector.tensor_tensor(out=rsf, in0=rsf, in1=ve, op=Alu.mult)
                nc.vector.tensor_scalar(out=rsf, in0=rsf, scalar1=-0.5,
                                        scalar2=1.5, op0=Alu.mult, op1=Alu.add)
                nc.vector.tensor_tensor(out=rsf, in0=rsf, in1=y0, op=Alu.mult)
                for i in range(4):
                    nt = 4 * s + i
                    qk_sb = qkn[i]
                    mean_bc = mv[:, i, :, 0:1].broadcast_to([128, 4, D])
                    rstd_bc = rstd[:, i:i + 1, :].rearrange("p o g -> p g o").broadcast_to([128, 4, D])
                    # normalize on the (otherwise idle) GpSimd engine to keep
                    # DVE off the critical path
                    nc.gpsimd.tensor_tensor(
                        out=qk_sb.rearrange("p (g d) -> p g d", g=4), in0=qk_sb.rearrange("p (g d) -> p g d", g=4),
                        in1=mean_bc, op=Alu.subtract)
                    nc.gpsimd.tensor_tensor(
                        out=qk_sb.rearrange("p (g d) -> p g d", g=4), in0=qk_sb.rearrange("p (g d) -> p g d", g=4),
                        in1=rstd_bc, op=Alu.mult)
                    # PE transpose, then PSUM evacuation with the LN affine
                    # (per-d scale/bias) folded into the tensor_scalar
                    pt_ps = psT.tile([128, 2, 128], mdt, tag="pt_ps",
                                     name=f"pt_ps{pair}_{nt}")
                    nc.tensor.transpose(pt_ps[:, 0, :], qk_sb[:, 0:128], identity)
                    nc.tensor.transpose(pt_ps[:, 1, :], qk_sb[:, 128:256], identity)
                    nc.vector.tensor_scalar(
                        out=q2[:, pair, nt * 128:(nt + 1) * 128],
                        in0=pt_ps[:, 0, :],
                        scalar1=aff_sb[:, 0:1], scalar2=aff_sb[:, 1:2],
                        op0=Alu.mult, op1=Alu.add)
                    nc.vector.tensor_scalar(
                        out=k2[:, pair, nt * 128:(nt + 1) * 128],
                        in0=pt_ps[:, 1, :],
                        scalar1=aff_sb[:, 2:3], scalar2=aff_sb[:, 3:4],
                        op0=Alu.mult, op1=Alu.add)

            # =========== stage B: one block = 1 jt tile of (pair, iq) ===========
            ps_o = {}
            psO_cur = [None]

            def emit_B_block(pair, iq, jt):
                if jt == 0:
                    for hp in range(2):
                        ps_o[hp] = psO_cur[0].tile([65, IQ], f32, tag=f"ps_o{hp}",
                                                   name=f"ps_o{pair}_{iq}_{hp}")
                ps_s = psS.tile([128, 2, IQ], f32, tag="ps_s",
                                name=f"ps_s{pair}_{iq}_{jt}")
                for hp in range(2):
                    po = hp * 64
                    nc.tensor.matmul(
                        ps_s[:, hp, :],
                        k2[po:po + 64, pair, jt * 128:(jt + 1) * 128],
                        q2[po:po + 64, pair, iq * IQ:(iq + 1) * IQ],
                        start=True, stop=True)
                pt = ptp.tile([128, 2, IQ], mdt, tag="pt",
                              name=f"pt{pair}_{iq}_{jt}")
                nc.scalar.activation(out=pt.rearrange("p a i -> p (a i)"),
                                     in_=ps_s.rearrange("p a i -> p (a i)"),
                                     func=Exp, scale=0.125)
                for hp in range(2):
                    nc.tensor.matmul(
                        ps_o[hp],
                        v_all[:, pair, jt, hp, :],
                        pt[:, hp, :],
                        start=(jt == 0), stop=(jt == 15))

            def emit_B_norm(pair, iq, last=False):
                for hp in range(2):
                    slot = 8 * pair + 2 * iq + hp
                    oe = nrm.tile([65, IQ], f32, tag="oe", name=f"oe{pair}_{iq}_{hp}")
                    nc.vector.tensor_copy(out=oe, in_=ps_o[hp])
                    # reciprocal in place on the single sumexp row, then one
                    # DRAM bounce for the partition-broadcast
                    nc.vector.reciprocal(out=oe[64:65, :], in_=oe[64:65, :])
                    nc.sync.dma_start(out=r_dram2[slot:slot + 1, :], in_=oe[64:65, :])
                    r_slot = r_dram2[slot, :]
                    r_bc = nrm.tile([64, IQ], f32, tag="r_bc")
                    nc.sync.dma_start(
                        out=r_bc,
                        in_=AP(tensor=r_slot.tensor, offset=r_slot.offset,
                               ap=[[0, 64], [1, IQ]]))
                    eng = nc.vector if last else nc.gpsimd
                    eng.tensor_tensor(
                        out=outT2[hp * 64:(hp + 1) * 64, pair, iq * IQ:(iq + 1) * IQ],
                        in0=oe[0:64, :], in1=r_bc, op=Alu.mult)
                nc.gpsimd.dma_start(
                    out=cc_in[pair][2 * iq:2 * iq + 2, :, :].rearrange("s d i -> d s i"),
                    in_=outT2[:, pair, iq * IQ:(iq + 1) * IQ]
                        .rearrange("d (s i) -> d s i", s=2))

            # =========== stage C ===========
            at_t = {}

            def emit_C_loads(phase, bb):
                # chunks kc = phase, phase+2, ... live in cc_out[phase] slots
                # [4*bb, 4*bb+4): one contiguous DMA per batch
                at = nrm.tile([128, 4, 256], mdt, tag=f"at{phase}_{bb}")
                nc.sync.dma_start(
                    out=at,
                    in_=cc_out[phase][4 * bb:4 * bb + 4].rearrange("s d i -> d s i"))
                at_t[(phase, bb)] = at

            def emit_C_block(phase, psC, bb, mt):
                ps_c = psC.tile([128, C], f32, tag="ps_c", name=f"ps_c{phase}_{bb}_{mt}")
                at = at_t[(phase, bb)]
                for ki in range(4):
                    for nk in range(2):
                        nc.tensor.matmul(
                            ps_c[:, nk * 512:(nk + 1) * 512],
                            at[:, ki, mt * 128:(mt + 1) * 128],
                            wp_sb[:, 2 * ki + phase, nk * 512:(nk + 1) * 512],
                            start=(ki == 0), stop=(ki == 3))
                idx = bb * 2 + mt
                if phase == 0:
                    nc.vector.tensor_tensor(out=o_sb[:, idx, :], in0=ps_c,
                                            in1=bproj_bc, op=Alu.add)
                else:
                    oo = nrm.tile([128, C], f32, tag="oo")
                    nc.vector.tensor_tensor(out=oo, in0=ps_c, in1=o_sb[:, idx, :],
                                            op=Alu.add)
                    nc.sync.dma_start(out=out_d.ap()[bb, mt * 128:(mt + 1) * 128, :],
                                      in_=oo)

            # =========== schedule ===========
            psO_cur[0] = psO
            # lead-in: pair-0 stage A supers feeding iq0 blocks progressively
            for s in range(4):
                for nt in range(4 * s, 4 * s + 4):
                    emit_A_mm(0, nt)
                emit_A_fin(0, s)
                for jt in range(4 * s, 4 * s + 4):
                    emit_B_block(0, 0, jt)
            emit_B_norm(0, 0)

            # deferred weight loads for stage C (sync queue is quiet mid-B0)
            for kc in range(8):
                nc.sync.dma_start(out=wp_sb[:, kc, :],
                                  in_=wproj_d.ap()[kc * 128:(kc + 1) * 128, :])
            nc.sync.dma_start(out=bproj_bc, in_=bcast(bproj_d, 128, C))

            # rest of B(0) with A(1) interleaved one n-tile at a time so the
            # PE stream never runs more than ~8 consecutive non-score matmuls
            a1_sched = {}
            for k in range(16):
                a1_sched.setdefault(k * 38 // 16, []).append(("mm", k))
                if k % 4 == 3:
                    a1_sched[k * 38 // 16].append(("fin", k // 4))
            blocks0 = [(iq, jt) for iq in (1, 2, 3) for jt in range(16)]
            for i, (iq, jt) in enumerate(blocks0):
                for kind, arg in a1_sched.get(i, []):
                    if kind == "mm":
                        emit_A_mm(1, arg)
                    else:
                        emit_A_fin(1, arg)
                emit_B_block(0, iq, jt)
                if jt == 15:
                    emit_B_norm(0, iq)
            actx.close()  # free stage-A + B(0) ps_o PSUM banks
            octx = ExitStack()
            psO2 = octx.enter_context(tc.tile_pool(name="psO2", bufs=2, space="PSUM"))
            psO_cur[0] = psO2

            nc.gpsimd.collective_compute(
                "AllToAll", Alu.bypass, replica_groups=groups,
                ins=[cc_in[0].opt()], outs=[cc_out[0].opt()])

            # B(1) with double-buffered ps_o so each iq's attn@V can start
            # before the previous iq's accumulator is evacuated
            blocks1 = [(iq, jt) for iq in range(4) for jt in range(16)]
            for i, (iq, jt) in enumerate(blocks1):
                emit_B_block(1, iq, jt)
                if jt == 15:
                    emit_B_norm(1, iq, last=(iq == 3))
            octx.close()
            psC = ctx.enter_context(tc.tile_pool(name="psC", bufs=1, space="PSUM"))

            nc.gpsimd.collective_compute(
                "AllToAll", Alu.bypass, replica_groups=groups,
                ins=[cc_in[1].opt()], outs=[cc_out[1].opt()])

            # C0 (pair-0 contraction half) overlaps the A2A#1 latency.
            # Everything gated on A2A#0 (incl. its at-loads) is emitted only
            # here: emitting it earlier lets the collective dependency leak
            # into the B(1) pipeline through shared DMA semaphore lanes.
            for bb in range(B):
                emit_C_loads(0, bb)
            for bb in range(B):
                for mt in range(2):
                    emit_C_block(0, psC, bb, mt)
            for bb in range(B):
                emit_C_loads(1, bb)
            for bb in range(B):
                for mt in range(2):
                    emit_C_block(1, psC, bb, mt)

    nc.compile()
    return nc


def kernel(**inputs):
    from concourse.bass_utils import run_bass_kernel_spmd
    import ml_dtypes

    trace = os.environ.get("KERNEL_TRACE", "0") == "1"
    if trace:
        _install_trace_shim()

    if "nc" not in _CACHE:
        _CACHE["nc"] = _build()
    nc = _CACHE["nc"]

    mnp = ml_dtypes.bfloat16

    x = np.asarray(inputs["x"], dtype=np.float32)
    w_qkv = np.asarray(inputs["w_qkv"], dtype=np.float32)
    b_qkv = np.asarray(inputs["b_qkv"], dtype=np.float32)
    w_proj = np.asarray(inputs["w_proj"], dtype=np.float32)
    b_proj = np.asarray(inputs["b_proj"], dtype=np.float32)
    q_scale = np.asarray(inputs["q_scale"], dtype=np.float32)
    q_bias = np.asarray(inputs["q_bias"], dtype=np.float32)
    k_scale = np.asarray(inputs["k_scale"], dtype=np.float32)
    k_bias = np.asarray(inputs["k_bias"], dtype=np.float32)

    aff = np.stack([np.tile(q_scale, 2), np.tile(q_bias, 2),
                    np.tile(k_scale, 2), np.tile(k_bias, 2)]).astype(np.float32)
    wproj_m = np.ascontiguousarray(w_proj.astype(mnp))

    in_maps = []
    for c in range(N_CORES):
        b, r = divmod(c, 4)
        w_all = np.empty((C, 2, 384), dtype=np.float32)
        bias_all = np.empty((2, 384), dtype=np.float32)
        for p in range(2):
            hs = slice((4 * r + 2 * p) * D, (4 * r + 2 * p) * D + 128)
            w_all[:, p, 0:128] = w_qkv[:, 0 * C:1 * C][:, hs]
            w_all[:, p, 128:256] = w_qkv[:, 1 * C:2 * C][:, hs]
            w_all[:, p, 256:384] = w_qkv[:, 2 * C:3 * C][:, hs]
            bias_all[p, 0:128] = b_qkv[0 * C:1 * C][hs]
            bias_all[p, 128:256] = b_qkv[1 * C:2 * C][hs]
            bias_all[p, 256:384] = b_qkv[2 * C:3 * C][hs]
        in_maps.append({
            "xT": np.ascontiguousarray(x[b].T.astype(mnp)),
            "w_all": np.ascontiguousarray(w_all.astype(mnp)),
            "bias_all": bias_all,
            "aff": aff,
            "wproj": wproj_m,
            "bproj": b_proj,
        })

    res = run_bass_kernel_spmd(nc, in_maps, core_ids=list(range(N_CORES)),
                               trace=trace)
    _CACHE["last_result"] = res

    out = np.empty((B, N, C), dtype=np.float32)
    for c in range(N_CORES):
        out[:, c * 256:(c + 1) * 256, :] = res.results[c]["out_part"]
    return out
